# revision 61
# baseline (speedup 1.0000x reference)
"""Discounted cumsum (y[b,h,t,d] = x[b,h,t,d] + gamma[h] * y[b,h,t-1,d]) on 8 trn2 cores.

Pure data parallelism: 64 (b,h) pairs, 8 per core. The per-pair recurrence runs
on two engines (the DVE scan instruction is fixed at ~2 cycles/elem, so Vector
alone caps at ~69 us/core; PE matmuls run ~1.1 ns/column under the hardware
power throttle, so a PE pair costs ~2x a DVE pair):

- 5 pairs fully on the Vector engine: layout [d=128 partitions, s=4096 free];
  the whole recurrence is ONE tensor_tensor_scan (state = gamma*state + x,
  fp32 internal state). gamma stays EXACT fp32 via a stride-0 broadcast AP -
  a rounded gamma would be raised to the power t by the recurrence, amplifying
  its error by ~t, while independently-rounded coefficient TABLES (PE path)
  and fp16 data are fine at the 2e-2 gate.

- 2 pairs fully on the Tensor engine (blocked parallel scan, single fp16):
  layout [t-within-block=128 partitions, 32 blocks x 128 d free]. Block sums
  r_k from u=gamma^(127-s) matmuls, carries C_k from one gamma^128-power
  matmul, carry injection as K=1 matmuls of gamma^(t+1) x C accumulated onto
  the within-block scan matmuls A[s,t]=gamma^(t-s) in PSUM. The scan matmuls
  go FIRST in each PSUM bank (start=True resets the whole bank region) and
  both scans of a bank precede the carry-dependent injects so a late carry
  never stalls them.

- 1 HYBRID pair: PE computes the first S/2 with the blocked scan (it needs no
  cross-engine input), the boundary state y[2047] is pulled from row 127 of
  the last block, PE-transposed to a [d,1] column, and handed to a final
  half-length DVE scan as its `initial`. This rebalances 5.5 DVE-pair-
  equivalents vs 2.5 PE, which is the measured-rate optimum.

Engine streams execute in program order and a DMA instruction's semaphore
wait stalls both its engine and its ring FIFO, so phases are emitted in an
order that keeps every stream stall-free:
- sync ring: x loads only (outputs on this ring measurably slow the DVE scan).
- scalar ring: the block-sum scatter + carry gather transposes (enqueued the
  moment their data is ready) and ALL outputs.
- gpsimd ring: constants only (its software DGE has ~6+ us latency).
- PE stream: each pair's block sums are emitted before the previous pair's
  carry-dependent half, so the in-order tensor stream always has work while
  a carry chain resolves (the scatter/gather semaphore round trip is ~8 us).

I/O is fp16 both ways (absmax error ~1e-2 against an absolute budget of 0.2);
all transposes are host-side so every HBM DMA is contiguous lines >= 4KB.
"""

import numpy as np

B, H, S, D = 4, 16, 4096, 128
T = 128          # block length (PE path matmul contraction dim)
KB = S // T      # 32 blocks
NG = 4           # blocks per matmul group (4*128 = 512 moving columns)
G = KB // NG     # 8 groups per pair
NCORES = 8
PAIRS = (B * H) // NCORES  # 8 pair-slots per core
NV = 5           # pairs fully on the Vector scan path
NPF = 2          # pairs fully on the Tensor (PE) path; +1 hybrid pair
S2 = S // 2
KB2 = KB // 2
G2 = G // 2

_nc_cache = {}


def _build_program():
    if "nc" in _nc_cache:
        return _nc_cache["nc"]

    import concourse.bass as bass
    import concourse.mybir as mybir
    from concourse.tile import TileContext

    f32 = mybir.dt.float32
    fp16 = mybir.dt.float16

    nc = bass.Bass(trn_type="TRN2")

    NP = NPF + 1  # PE coefficient slots: 2 full + 1 hybrid

    xv_d = nc.declare_dram_parameter("xv", [NV, D, S], fp16, isOutput=False)
    g_d = nc.declare_dram_parameter("g", [D, NV + 1], f32, isOutput=False)
    yv_d = nc.declare_dram_parameter("yv", [NV, D, S], fp16, isOutput=True)

    xp_d = nc.declare_dram_parameter("xp", [NPF, T, KB * D], fp16, isOutput=False)
    xph_d = nc.declare_dram_parameter("xph", [T, KB2 * D], fp16, isOutput=False)
    xvh_d = nc.declare_dram_parameter("xvh", [D, S2], fp16, isOutput=False)
    A_d = nc.declare_dram_parameter("A_all", [T, NP * T], fp16, isOutput=False)
    u_d = nc.declare_dram_parameter("u_all", [T, NP], fp16, isOutput=False)
    gv_d = nc.declare_dram_parameter("gv_all", [1, NP * T], fp16, isOutput=False)
    GT_d = nc.declare_dram_parameter("GT_all", [KB, NP * KB], fp16, isOutput=False)
    yp_d = nc.declare_dram_parameter("yp", [NPF, T, KB * D], fp16, isOutput=True)
    yph_d = nc.declare_dram_parameter("yph", [T, KB2 * D], fp16, isOutput=True)
    yvh_d = nc.declare_dram_parameter("yvh", [D, S2], fp16, isOutput=True)

    mult, add = mybir.AluOpType.mult, mybir.AluOpType.add

    with TileContext(nc) as tc:
        with (
            tc.tile_pool(name="const", bufs=1) as cpool,
            tc.tile_pool(name="xvin", bufs=3) as xvpool,
            tc.tile_pool(name="yvout", bufs=3) as yvpool,
            tc.tile_pool(name="xpin", bufs=3) as xppool,
            tc.tile_pool(name="ypout", bufs=2) as yppool,
            tc.tile_pool(name="rfl", bufs=2) as rfpool,
            tc.tile_pool(name="r32", bufs=2) as r32pool,
            tc.tile_pool(name="c32", bufs=2) as c32pool,
            tc.tile_pool(name="cfl", bufs=2) as cfpool,
            tc.tile_pool(name="grp_ps", bufs=2, space="PSUM") as gp_pool,
            tc.tile_pool(name="sum_ps", bufs=2, space="PSUM") as sp_pool,
            tc.tile_pool(name="c_ps", bufs=1, space="PSUM") as cp_pool,
        ):
            gc = cpool.tile([D, NV + 1], f32, tag="gc")
            nc.gpsimd.dma_start(out=gc[:], in_=g_d[:])
            Ac = cpool.tile([T, NP * T], fp16, tag="Ac")
            uc = cpool.tile([T, NP], fp16, tag="uc")
            gvc = cpool.tile([1, NP * T], fp16, tag="gvc")
            GTc = cpool.tile([KB, NP * KB], fp16, tag="GTc")
            nc.gpsimd.dma_start(out=Ac[:], in_=A_d[:])
            nc.gpsimd.dma_start(out=uc[:], in_=u_d[:])
            nc.gpsimd.dma_start(out=gvc[:], in_=gv_d[:])
            nc.gpsimd.dma_start(out=GTc[:], in_=GT_d[:])
            # hybrid boundary state, written in pe_phase_gt via scalar copy
            c7 = cpool.tile([D, 1], f32, tag="c7")

            v_state = {}
            pe_state = {}

            def v_scan(v):
                X = xvpool.tile([D, S], fp16, tag="Xv")
                nc.sync.dma_start(out=X[:], in_=xv_d[v])
                Y = yvpool.tile([D, S], fp16, tag="Yv")
                nc.vector.tensor_tensor_scan(
                    out=Y[:], data0=gc[:, v : v + 1].broadcast_to([D, S]),
                    data1=X[:], initial=0.0, op0=mult, op1=add,
                )
                v_state[v] = Y

            def v_out(v, split_rings=False):
                Y = v_state.pop(v)
                if split_rings:
                    # final output: halves on different rings move in parallel
                    nc.sync.dma_start(out=yv_d[v][:, :S2], in_=Y[:, :S2])
                    nc.scalar.dma_start(out=yv_d[v][:, S2:], in_=Y[:, S2:])
                else:
                    nc.scalar.dma_start(out=yv_d[v], in_=Y[:])

            def v_scan_h(_):
                X = xvpool.tile([D, S2], fp16, tag="Xvh")
                nc.sync.dma_start(out=X[:], in_=xvh_d[:])
                Y = yvpool.tile([D, S2], fp16, tag="Yvh")
                nc.vector.tensor_tensor_scan(
                    out=Y[:], data0=gc[:, NV : NV + 1].broadcast_to([D, S2]),
                    data1=X[:], initial=c7[:, 0:1], op0=mult, op1=add,
                )
                v_state["h"] = Y

            def v_out_h(_):
                Y = v_state.pop("h")
                nc.scalar.dma_start(out=yvh_d[:], in_=Y[:])

            def pe_phase_a(q):
                """Block sums for a PE slot; q=2 is the half-length hybrid."""
                nk = KB if q < NPF else KB2
                ng = G if q < NPF else G2
                Xf = xppool.tile([T, KB * D], fp16, tag="Xp")
                X = Xf[:, 0 : nk * D]
                nc.sync.dma_start(
                    out=X, in_=xp_d[q] if q < NPF else xph_d[:]
                )
                Rff = rfpool.tile([1, KB * D], fp16, tag="Rf")
                Rflat = Rff[:, 0 : nk * D]
                for g in range(ng):
                    sl = slice(g * NG * D, (g + 1) * NG * D)
                    rp = sp_pool.tile([1, NG * D], f32, tag="rp")
                    nc.tensor.matmul(
                        rp[:], lhsT=uc[:, q : q + 1], rhs=X[:, sl],
                        start=True, stop=True,
                    )
                    nc.scalar.copy(out=Rflat[:, sl], in_=rp[:])
                R32f = r32pool.tile([KB, D], fp16, tag="R32")
                R32 = R32f[0:nk, :]
                nc.scalar.dma_start(out=R32, in_=Rflat)
                pe_state[q] = (X, R32)

            def pe_phase_gt(q):
                nk = KB if q < NPF else KB2
                X, R32 = pe_state.pop(q)
                cpf = cp_pool.tile([KB, D], f32, tag="cp")
                cp = cpf[0:nk, :]
                nc.tensor.matmul(
                    cp[:], lhsT=GTc[0:nk, q * KB : q * KB + nk], rhs=R32[:],
                    start=True, stop=True,
                )
                if q == NPF:
                    # boundary state y[S2-1, d] = sum_j gamma^(128(15-j)) r_j
                    # with R32 as the STATIONARY so the result lands as the
                    # [d, 1] column the DVE scan needs for `initial`
                    tp = cp_pool.tile([D, 1], f32, tag="tp")
                    nc.tensor.matmul(
                        tp[:], lhsT=R32[:],
                        rhs=GTc[0:nk, q * KB + nk : q * KB + nk + 1],
                        start=True, stop=True,
                    )
                    nc.scalar.copy(out=c7[:], in_=tp[:])
                C32f = c32pool.tile([KB, D], fp16, tag="C32")
                C32 = C32f[0:nk, :]
                nc.scalar.copy(out=C32, in_=cp[:])
                cff = cfpool.tile([1, KB * D], fp16, tag="cfh")
                cfh = cff[:, 0 : nk * D]
                nc.scalar.dma_start(out=cfh, in_=C32)
                pe_state[q] = (X, cfh)

            def pe_phase_scan(q):
                nk = KB if q < NPF else KB2
                ng = G if q < NPF else G2
                X, cfh = pe_state.pop(q)
                Ysf = yppool.tile([T, KB * D], fp16, tag="Ys")
                Ys = Ysf[:, 0 : nk * D]
                Aq = Ac[:, q * T : (q + 1) * T]
                gvq = gvc[:, q * T : (q + 1) * T]
                for g2 in range(ng // 2):
                    sl2 = slice(g2 * 2 * NG * D, (g2 + 1) * 2 * NG * D)
                    grp = gp_pool.tile([T, 2 * NG * D], f32, tag="grp")
                    for h in range(2):
                        g = g2 * 2 + h
                        nc.tensor.matmul(
                            grp[:, h * NG * D : (h + 1) * NG * D],
                            lhsT=Aq,
                            rhs=X[:, g * NG * D : (g + 1) * NG * D],
                            start=True, stop=False,
                            skip_group_check=True,
                        )
                    for h in range(2):
                        g = g2 * 2 + h
                        nc.tensor.matmul(
                            grp[:, h * NG * D : (h + 1) * NG * D],
                            lhsT=gvq,
                            rhs=cfh[:, g * NG * D : (g + 1) * NG * D],
                            start=False, stop=True,
                            skip_group_check=True,
                        )
                    nc.scalar.copy(out=Ys[:, sl2], in_=grp[:])
                    if g2 % 2 == 1:
                        slo = slice((g2 - 1) * 2 * NG * D, (g2 + 1) * 2 * NG * D)
                        # sync ring: idle once the x loads finish (~t=37)
                        if q < NPF:
                            nc.sync.dma_start(
                                out=yp_d[q, :, slo], in_=Ys[:, slo]
                            )
                        else:
                            nc.sync.dma_start(out=yph_d[:, slo], in_=Ys[:, slo])


            phases = {
                "vs": v_scan,
                "vo": v_out,
                "vsh": v_scan_h,
                "voh": v_out_h,
                "a": pe_phase_a,
                "gt": pe_phase_gt,
                "sc": pe_phase_scan,
            }
            for kind, idx, *kw in [
                ("vs", 0), ("a", 2), ("a", 0), ("vs", 1), ("a", 1),
                ("gt", 2), ("vs", 2), ("vo", 0), ("sc", 2), ("gt", 0),
                ("vs", 3), ("vo", 1), ("sc", 0), ("gt", 1), ("vo", 2),
                ("vs", 4), ("sc", 1), ("vo", 3), ("vsh", 0), ("voh", 0),
                ("vo", 4, {"split_rings": True}),
            ]:
                phases[kind](idx, **(kw[0] if kw else {}))

    # Walrus allows 1 sync wait on engine instructions / 2 on DMAs; move
    # excess waits onto InstEventSemaphore carriers.
    import bass_rust

    bass_rust.generate_event_semaphores(nc)

    _nc_cache["nc"] = nc
    return nc


def _pe_constants(g):
    """fp16 coefficient tables from float64 gamma powers."""
    pw = np.power(g, np.arange(S, dtype=np.float64))
    t_idx = np.arange(T)
    t_minus_s = t_idx[None, :] - t_idx[:, None]
    A = np.where(t_minus_s >= 0, pw[np.clip(t_minus_s, 0, None)], 0.0)
    u = pw[127 - t_idx]
    gv = pw[t_idx + 1]
    pw128 = np.power(pw[T], np.arange(KB, dtype=np.float64))
    k_minus_j = np.arange(KB)[None, :] - 1 - np.arange(KB)[:, None]
    GT = np.where(k_minus_j >= 0, pw128[np.clip(k_minus_j, 0, None)], 0.0)
    return A, u, gv, GT


def _pe_layout(xpair):
    """[S', D] f32 -> [T, (block, d)] fp16 scan layout."""
    nk = xpair.shape[0] // T
    return (
        xpair.reshape(nk, T, D).transpose(1, 0, 2).reshape(T, nk * D)
        .astype(np.float16)
    )


def _make_in_maps(tensor, gamma):
    x = np.asarray(tensor, dtype=np.float32).reshape(B * H, S, D)
    gam = np.asarray(gamma, dtype=np.float32).reshape(H)
    NP = NPF + 1

    in_maps = []
    for c in range(NCORES):
        pids = [c * PAIRS + p for p in range(PAIRS)]
        xv = np.empty((NV, D, S), np.float16)
        gcol = np.empty((D, NV + 1), np.float32)
        for v in range(NV):
            pid = pids[v]
            xv[v] = x[pid].T.astype(np.float16)
            gcol[:, v] = gam[pid % H]
        hid = pids[NV + NPF]  # the hybrid pair
        gcol[:, NV] = gam[hid % H]
        xp = np.empty((NPF, T, KB * D), np.float16)
        A_all = np.empty((T, NP * T), np.float16)
        u_all = np.empty((T, NP), np.float16)
        gv_all = np.empty((1, NP * T), np.float16)
        GT_all = np.zeros((KB, NP * KB), np.float16)
        for q in range(NP):
            pid = pids[NV + q]
            if q < NPF:
                xp[q] = _pe_layout(x[pid])
            A, u, gv, GT = _pe_constants(float(gam[pid % H]))
            A_all[:, q * T : (q + 1) * T] = A.astype(np.float16)
            u_all[:, q] = u.astype(np.float16)
            gv_all[0, q * T : (q + 1) * T] = gv.astype(np.float16)
            GT_all[:, q * KB : (q + 1) * KB] = GT.astype(np.float16)
            if q == NPF:
                # boundary-state weights: y[S2-1] = sum_j g^(128(15-j)) r_j
                gq = np.float64(gam[pid % H])
                w16 = np.power(
                    np.power(gq, T), KB2 - 1 - np.arange(KB2, dtype=np.float64)
                )
                GT_all[0:KB2, q * KB + KB2] = w16.astype(np.float16)
        in_maps.append(
            {
                "xv": xv,
                "g": gcol,
                "xp": xp,
                "xph": _pe_layout(x[hid][:S2]),
                "xvh": x[hid][S2:].T.astype(np.float16),
                "A_all": A_all,
                "u_all": u_all,
                "gv_all": gv_all,
                "GT_all": GT_all,
            }
        )
    return in_maps


def _gather_results(results):
    """Per-core result dicts -> full [B, H, S, D] f32 output."""
    y = np.empty((B * H, S, D), np.float32)
    for c in range(NCORES):
        r = results[c]
        yv = np.asarray(r["yv"])
        for v in range(NV):
            y[c * PAIRS + v] = yv[v].T
        yp = np.asarray(r["yp"])
        for q in range(NPF):
            y[c * PAIRS + NV + q] = (
                yp[q].reshape(T, KB, D).transpose(1, 0, 2).reshape(S, D)
            )
        hid = c * PAIRS + NV + NPF
        y[hid, :S2] = (
            np.asarray(r["yph"]).reshape(T, KB2, D).transpose(1, 0, 2)
            .reshape(S2, D)
        )
        y[hid, S2:] = np.asarray(r["yvh"]).T
    return y.reshape(B, H, S, D)


def kernel(tensor, gamma):
    from concourse.bass_utils import run_bass_kernel_spmd

    in_maps = _make_in_maps(tensor, gamma)
    nc = _build_program()
    res = run_bass_kernel_spmd(nc, in_maps, list(range(NCORES))).results
    return _gather_results(res)


# revision 63
# speedup vs baseline: 1.0052x; 1.0052x over previous
"""Discounted cumsum (y[b,h,t,d] = x[b,h,t,d] + gamma[h] * y[b,h,t-1,d]) on 8 trn2 cores.

Pure data parallelism: 64 (b,h) pairs, 8 per core. The per-pair recurrence runs
on two engines (the DVE scan instruction is fixed at ~2 cycles/elem, so Vector
alone caps at ~69 us/core; PE matmuls run ~1.1 ns/column under the hardware
power throttle, so a PE pair costs ~2x a DVE pair):

- 5 pairs fully on the Vector engine: layout [d=128 partitions, s=4096 free];
  the whole recurrence is ONE tensor_tensor_scan (state = gamma*state + x,
  fp32 internal state). gamma stays EXACT fp32 via a stride-0 broadcast AP -
  a rounded gamma would be raised to the power t by the recurrence, amplifying
  its error by ~t, while independently-rounded coefficient TABLES (PE path)
  and fp16 data are fine at the 2e-2 gate.

- 2 pairs fully on the Tensor engine (blocked parallel scan, single fp16):
  layout [t-within-block=128 partitions, 32 blocks x 128 d free]. Block sums
  r_k from u=gamma^(127-s) matmuls, carries C_k from one gamma^128-power
  matmul, carry injection as K=1 matmuls of gamma^(t+1) x C accumulated onto
  the within-block scan matmuls A[s,t]=gamma^(t-s) in PSUM. The scan matmuls
  go FIRST in each PSUM bank (start=True resets the whole bank region) and
  both scans of a bank precede the carry-dependent injects so a late carry
  never stalls them.

- 1 HYBRID pair: PE computes the first S/2 with the blocked scan (it needs no
  cross-engine input), the boundary state y[2047] is pulled from row 127 of
  the last block, PE-transposed to a [d,1] column, and handed to a final
  half-length DVE scan as its `initial`. This rebalances 5.5 DVE-pair-
  equivalents vs 2.5 PE, which is the measured-rate optimum.

Engine streams execute in program order and a DMA instruction's semaphore
wait stalls both its engine and its ring FIFO, so phases are emitted in an
order that keeps every stream stall-free:
- sync ring: x loads only (outputs on this ring measurably slow the DVE scan).
- scalar ring: the block-sum scatter + carry gather transposes (enqueued the
  moment their data is ready) and ALL outputs.
- gpsimd ring: constants only (its software DGE has ~6+ us latency).
- PE stream: each pair's block sums are emitted before the previous pair's
  carry-dependent half, so the in-order tensor stream always has work while
  a carry chain resolves (the scatter/gather semaphore round trip is ~8 us).

I/O is fp16 both ways (absmax error ~1e-2 against an absolute budget of 0.2);
all transposes are host-side so every HBM DMA is contiguous lines >= 4KB.
"""

import numpy as np

B, H, S, D = 4, 16, 4096, 128
T = 128          # block length (PE path matmul contraction dim)
KB = S // T      # 32 blocks
NG = 4           # blocks per matmul group (4*128 = 512 moving columns)
G = KB // NG     # 8 groups per pair
NCORES = 8
PAIRS = (B * H) // NCORES  # 8 pair-slots per core
NV = 5           # pairs fully on the Vector scan path
NPF = 2          # pairs fully on the Tensor (PE) path; +1 hybrid pair
S2 = S // 2
KB2 = KB // 2
G2 = G // 2

_nc_cache = {}


def _build_program():
    if "nc" in _nc_cache:
        return _nc_cache["nc"]

    import concourse.bass as bass
    import concourse.mybir as mybir
    from concourse.tile import TileContext

    f32 = mybir.dt.float32
    fp16 = mybir.dt.float16

    nc = bass.Bass(trn_type="TRN2")

    NP = NPF + 1  # PE coefficient slots: 2 full + 1 hybrid

    xv_d = nc.declare_dram_parameter("xv", [NV, D, S], fp16, isOutput=False)
    g_d = nc.declare_dram_parameter("g", [D, NV + 1], f32, isOutput=False)
    yv_d = nc.declare_dram_parameter("yv", [NV, D, S], fp16, isOutput=True)

    xp_d = nc.declare_dram_parameter("xp", [NPF, T, KB * D], fp16, isOutput=False)
    xph_d = nc.declare_dram_parameter("xph", [T, KB2 * D], fp16, isOutput=False)
    xvh_d = nc.declare_dram_parameter("xvh", [D, S2], fp16, isOutput=False)
    A_d = nc.declare_dram_parameter("A_all", [T, NP * T], fp16, isOutput=False)
    u_d = nc.declare_dram_parameter("u_all", [T, NP], fp16, isOutput=False)
    gv_d = nc.declare_dram_parameter("gv_all", [1, NP * T], fp16, isOutput=False)
    GT_d = nc.declare_dram_parameter("GT_all", [KB, NP * KB], fp16, isOutput=False)
    yp_d = nc.declare_dram_parameter("yp", [NPF, T, KB * D], fp16, isOutput=True)
    yph_d = nc.declare_dram_parameter("yph", [T, KB2 * D], fp16, isOutput=True)
    yvh_d = nc.declare_dram_parameter("yvh", [D, S2], fp16, isOutput=True)

    mult, add = mybir.AluOpType.mult, mybir.AluOpType.add

    with TileContext(nc) as tc:
        with (
            tc.tile_pool(name="const", bufs=1) as cpool,
            tc.tile_pool(name="xvin", bufs=3) as xvpool,
            tc.tile_pool(name="yvout", bufs=3) as yvpool,
            tc.tile_pool(name="xpin", bufs=3) as xppool,
            tc.tile_pool(name="ypout", bufs=2) as yppool,
            tc.tile_pool(name="rfl", bufs=2) as rfpool,
            tc.tile_pool(name="r32", bufs=2) as r32pool,
            tc.tile_pool(name="c32", bufs=2) as c32pool,
            tc.tile_pool(name="cfl", bufs=2) as cfpool,
            tc.tile_pool(name="grp_ps", bufs=2, space="PSUM") as gp_pool,
            tc.tile_pool(name="sum_ps", bufs=2, space="PSUM") as sp_pool,
            tc.tile_pool(name="c_ps", bufs=1, space="PSUM") as cp_pool,
        ):
            gc = cpool.tile([D, NV + 1], f32, tag="gc")
            nc.gpsimd.dma_start(out=gc[:], in_=g_d[:])
            Ac = cpool.tile([T, NP * T], fp16, tag="Ac")
            uc = cpool.tile([T, NP], fp16, tag="uc")
            gvc = cpool.tile([1, NP * T], fp16, tag="gvc")
            GTc = cpool.tile([KB, NP * KB], fp16, tag="GTc")
            nc.gpsimd.dma_start(out=Ac[:], in_=A_d[:])
            nc.gpsimd.dma_start(out=uc[:], in_=u_d[:])
            nc.gpsimd.dma_start(out=gvc[:], in_=gv_d[:])
            nc.gpsimd.dma_start(out=GTc[:], in_=GT_d[:])
            # hybrid boundary state, written in pe_phase_gt via scalar copy
            c7 = cpool.tile([D, 1], f32, tag="c7")

            v_state = {}
            pe_state = {}

            def v_scan(v):
                X = xvpool.tile([D, S], fp16, tag="Xv")
                nc.sync.dma_start(out=X[:], in_=xv_d[v])
                Y = yvpool.tile([D, S], fp16, tag="Yv")
                nc.vector.tensor_tensor_scan(
                    out=Y[:], data0=gc[:, v : v + 1].broadcast_to([D, S]),
                    data1=X[:], initial=0.0, op0=mult, op1=add,
                )
                v_state[v] = Y

            def v_out(v):
                Y = v_state.pop(v)
                nc.scalar.dma_start(out=yv_d[v], in_=Y[:])

            def v_scan_h(_):
                X = xvpool.tile([D, S2], fp16, tag="Xvh")
                nc.sync.dma_start(out=X[:], in_=xvh_d[:])
                Y = yvpool.tile([D, S2], fp16, tag="Yvh")
                nc.vector.tensor_tensor_scan(
                    out=Y[:], data0=gc[:, NV : NV + 1].broadcast_to([D, S2]),
                    data1=X[:], initial=c7[:, 0:1], op0=mult, op1=add,
                )
                v_state["h"] = Y

            def v_out_h(_):
                Y = v_state.pop("h")
                nc.scalar.dma_start(out=yvh_d[:], in_=Y[:])

            def pe_phase_a(q):
                """Block sums for a PE slot; q=2 is the half-length hybrid."""
                nk = KB if q < NPF else KB2
                ng = G if q < NPF else G2
                Xf = xppool.tile([T, KB * D], fp16, tag="Xp")
                X = Xf[:, 0 : nk * D]
                nc.sync.dma_start(
                    out=X, in_=xp_d[q] if q < NPF else xph_d[:]
                )
                Rff = rfpool.tile([1, KB * D], fp16, tag="Rf")
                Rflat = Rff[:, 0 : nk * D]
                for g in range(ng):
                    sl = slice(g * NG * D, (g + 1) * NG * D)
                    rp = sp_pool.tile([1, NG * D], f32, tag="rp")
                    nc.tensor.matmul(
                        rp[:], lhsT=uc[:, q : q + 1], rhs=X[:, sl],
                        start=True, stop=True,
                    )
                    nc.scalar.copy(out=Rflat[:, sl], in_=rp[:])
                R32f = r32pool.tile([KB, D], fp16, tag="R32")
                R32 = R32f[0:nk, :]
                nc.scalar.dma_start(out=R32, in_=Rflat)
                pe_state[q] = (X, R32)

            def pe_phase_gt(q):
                nk = KB if q < NPF else KB2
                X, R32 = pe_state.pop(q)
                cpf = cp_pool.tile([KB, D], f32, tag="cp")
                cp = cpf[0:nk, :]
                nc.tensor.matmul(
                    cp[:], lhsT=GTc[0:nk, q * KB : q * KB + nk], rhs=R32[:],
                    start=True, stop=True,
                )
                if q == NPF:
                    # boundary state y[S2-1, d] = sum_j gamma^(128(15-j)) r_j
                    # with R32 as the STATIONARY so the result lands as the
                    # [d, 1] column the DVE scan needs for `initial`
                    tp = cp_pool.tile([D, 1], f32, tag="tp")
                    nc.tensor.matmul(
                        tp[:], lhsT=R32[:],
                        rhs=GTc[0:nk, q * KB + nk : q * KB + nk + 1],
                        start=True, stop=True,
                    )
                    nc.scalar.copy(out=c7[:], in_=tp[:])
                C32f = c32pool.tile([KB, D], fp16, tag="C32")
                C32 = C32f[0:nk, :]
                nc.scalar.copy(out=C32, in_=cp[:])
                cff = cfpool.tile([1, KB * D], fp16, tag="cfh")
                cfh = cff[:, 0 : nk * D]
                nc.scalar.dma_start(out=cfh, in_=C32)
                pe_state[q] = (X, cfh)

            def pe_phase_scan(q):
                nk = KB if q < NPF else KB2
                ng = G if q < NPF else G2
                X, cfh = pe_state.pop(q)
                Ysf = yppool.tile([T, KB * D], fp16, tag="Ys")
                Ys = Ysf[:, 0 : nk * D]
                Aq = Ac[:, q * T : (q + 1) * T]
                gvq = gvc[:, q * T : (q + 1) * T]
                for g2 in range(ng // 2):
                    sl2 = slice(g2 * 2 * NG * D, (g2 + 1) * 2 * NG * D)
                    grp = gp_pool.tile([T, 2 * NG * D], f32, tag="grp")
                    for h in range(2):
                        g = g2 * 2 + h
                        nc.tensor.matmul(
                            grp[:, h * NG * D : (h + 1) * NG * D],
                            lhsT=Aq,
                            rhs=X[:, g * NG * D : (g + 1) * NG * D],
                            start=True, stop=False,
                            skip_group_check=True,
                        )
                    for h in range(2):
                        g = g2 * 2 + h
                        nc.tensor.matmul(
                            grp[:, h * NG * D : (h + 1) * NG * D],
                            lhsT=gvq,
                            rhs=cfh[:, g * NG * D : (g + 1) * NG * D],
                            start=False, stop=True,
                            skip_group_check=True,
                        )
                    nc.scalar.copy(out=Ys[:, sl2], in_=grp[:])
                    if g2 % 2 == 1:
                        slo = slice((g2 - 1) * 2 * NG * D, (g2 + 1) * 2 * NG * D)
                        # sync ring: idle once the x loads finish (~t=37)
                        if q < NPF:
                            nc.sync.dma_start(
                                out=yp_d[q, :, slo], in_=Ys[:, slo]
                            )
                        else:
                            nc.sync.dma_start(out=yph_d[:, slo], in_=Ys[:, slo])


            phases = {
                "vs": v_scan,
                "vo": v_out,
                "vsh": v_scan_h,
                "voh": v_out_h,
                "a": pe_phase_a,
                "gt": pe_phase_gt,
                "sc": pe_phase_scan,
            }
            for kind, idx in [
                ("vs", 0), ("a", 2), ("a", 0), ("vs", 1), ("a", 1),
                ("gt", 2), ("vs", 2), ("vo", 0), ("sc", 2), ("gt", 0),
                ("vs", 3), ("vo", 1), ("sc", 0), ("gt", 1), ("vo", 2),
                ("vs", 4), ("sc", 1), ("vo", 3), ("vsh", 0), ("voh", 0),
                ("vo", 4),
            ]:
                phases[kind](idx)

    # Walrus allows 1 sync wait on engine instructions / 2 on DMAs; move
    # excess waits onto InstEventSemaphore carriers.
    import bass_rust

    bass_rust.generate_event_semaphores(nc)

    _nc_cache["nc"] = nc
    return nc


def _pe_constants(g):
    """fp16 coefficient tables from float64 gamma powers."""
    pw = np.power(g, np.arange(S, dtype=np.float64))
    t_idx = np.arange(T)
    t_minus_s = t_idx[None, :] - t_idx[:, None]
    A = np.where(t_minus_s >= 0, pw[np.clip(t_minus_s, 0, None)], 0.0)
    u = pw[127 - t_idx]
    gv = pw[t_idx + 1]
    pw128 = np.power(pw[T], np.arange(KB, dtype=np.float64))
    k_minus_j = np.arange(KB)[None, :] - 1 - np.arange(KB)[:, None]
    GT = np.where(k_minus_j >= 0, pw128[np.clip(k_minus_j, 0, None)], 0.0)
    return A, u, gv, GT


def _pe_layout(xpair):
    """[S', D] f32 -> [T, (block, d)] fp16 scan layout."""
    nk = xpair.shape[0] // T
    return (
        xpair.reshape(nk, T, D).transpose(1, 0, 2).reshape(T, nk * D)
        .astype(np.float16)
    )


def _make_in_maps(tensor, gamma):
    x = np.asarray(tensor, dtype=np.float32).reshape(B * H, S, D)
    gam = np.asarray(gamma, dtype=np.float32).reshape(H)
    NP = NPF + 1

    in_maps = []
    for c in range(NCORES):
        pids = [c * PAIRS + p for p in range(PAIRS)]
        xv = np.empty((NV, D, S), np.float16)
        gcol = np.empty((D, NV + 1), np.float32)
        for v in range(NV):
            pid = pids[v]
            xv[v] = x[pid].T.astype(np.float16)
            gcol[:, v] = gam[pid % H]
        hid = pids[NV + NPF]  # the hybrid pair
        gcol[:, NV] = gam[hid % H]
        xp = np.empty((NPF, T, KB * D), np.float16)
        A_all = np.empty((T, NP * T), np.float16)
        u_all = np.empty((T, NP), np.float16)
        gv_all = np.empty((1, NP * T), np.float16)
        GT_all = np.zeros((KB, NP * KB), np.float16)
        for q in range(NP):
            pid = pids[NV + q]
            if q < NPF:
                xp[q] = _pe_layout(x[pid])
            A, u, gv, GT = _pe_constants(float(gam[pid % H]))
            A_all[:, q * T : (q + 1) * T] = A.astype(np.float16)
            u_all[:, q] = u.astype(np.float16)
            gv_all[0, q * T : (q + 1) * T] = gv.astype(np.float16)
            GT_all[:, q * KB : (q + 1) * KB] = GT.astype(np.float16)
            if q == NPF:
                # boundary-state weights: y[S2-1] = sum_j g^(128(15-j)) r_j
                gq = np.float64(gam[pid % H])
                w16 = np.power(
                    np.power(gq, T), KB2 - 1 - np.arange(KB2, dtype=np.float64)
                )
                GT_all[0:KB2, q * KB + KB2] = w16.astype(np.float16)
        in_maps.append(
            {
                "xv": xv,
                "g": gcol,
                "xp": xp,
                "xph": _pe_layout(x[hid][:S2]),
                "xvh": x[hid][S2:].T.astype(np.float16),
                "A_all": A_all,
                "u_all": u_all,
                "gv_all": gv_all,
                "GT_all": GT_all,
            }
        )
    return in_maps


def _gather_results(results):
    """Per-core result dicts -> full [B, H, S, D] f32 output."""
    y = np.empty((B * H, S, D), np.float32)
    for c in range(NCORES):
        r = results[c]
        yv = np.asarray(r["yv"])
        for v in range(NV):
            y[c * PAIRS + v] = yv[v].T
        yp = np.asarray(r["yp"])
        for q in range(NPF):
            y[c * PAIRS + NV + q] = (
                yp[q].reshape(T, KB, D).transpose(1, 0, 2).reshape(S, D)
            )
        hid = c * PAIRS + NV + NPF
        y[hid, :S2] = (
            np.asarray(r["yph"]).reshape(T, KB2, D).transpose(1, 0, 2)
            .reshape(S2, D)
        )
        y[hid, S2:] = np.asarray(r["yvh"]).T
    return y.reshape(B, H, S, D)


def kernel(tensor, gamma):
    from concourse.bass_utils import run_bass_kernel_spmd

    in_maps = _make_in_maps(tensor, gamma)
    nc = _build_program()
    res = run_bass_kernel_spmd(nc, in_maps, list(range(NCORES))).results
    return _gather_results(res)


# revision 65
# speedup vs baseline: 1.0448x; 1.0394x over previous
"""Discounted cumsum (y[b,h,t,d] = x[b,h,t,d] + gamma[h] * y[b,h,t-1,d]) on 8 trn2 cores.

Pure data parallelism: 64 (b,h) pairs, 8 per core. The per-pair recurrence runs
on two engines (the DVE scan instruction is fixed at ~2 cycles/elem, so Vector
alone caps at ~69 us/core; PE matmuls run ~1.1 ns/column under the hardware
power throttle, so a PE pair costs ~2x a DVE pair):

- 5 pairs fully on the Vector engine: layout [d=128 partitions, s=4096 free];
  the whole recurrence is ONE tensor_tensor_scan (state = gamma*state + x,
  fp32 internal state). gamma stays EXACT fp32 via a stride-0 broadcast AP -
  a rounded gamma would be raised to the power t by the recurrence, amplifying
  its error by ~t, while independently-rounded coefficient TABLES (PE path)
  and fp16 data are fine at the 2e-2 gate.

- 2 pairs fully on the Tensor engine (blocked parallel scan, single fp16):
  layout [t-within-block=128 partitions, 32 blocks x 128 d free]. Block sums
  r_k from u=gamma^(127-s) matmuls, carries C_k from one gamma^128-power
  matmul, carry injection as K=1 matmuls of gamma^(t+1) x C accumulated onto
  the within-block scan matmuls A[s,t]=gamma^(t-s) in PSUM. The scan matmuls
  go FIRST in each PSUM bank (start=True resets the whole bank region) and
  both scans of a bank precede the carry-dependent injects so a late carry
  never stalls them.

- 1 HYBRID pair: PE computes the first S/2 with the blocked scan (it needs no
  cross-engine input); the boundary state y[S/2-1] is computed from the block
  sums by one extra matmul with R32 as the STATIONARY operand (so the result
  lands directly as the [d,1] column a DVE scan needs for `initial`), and the
  second half runs as a final half-length DVE scan. This rebalances 5.5
  DVE-pair-equivalents vs 2.5 PE, which is the measured-rate optimum.

Engine streams execute in program order and a DMA instruction's semaphore
wait stalls both its engine and its ring FIFO, so phases are emitted in an
order that keeps every stream stall-free:
- sync ring: x loads, then the PE-path outputs (the ring is idle once the
  loads drain and the PE outputs become ready in that window).
- scalar ring: the block-sum scatter + carry gather transposes (enqueued the
  moment their data is ready) and the Vector-path outputs.
- gpsimd ring: constants only (its software DGE has ~6+ us latency).
- PE stream: each pair's block sums are emitted before the previous pair's
  carry-dependent half, so the in-order tensor stream always has work while
  a carry chain resolves (the scatter/gather semaphore round trip is ~8 us).

I/O is fp16 both ways (absmax error ~1e-2 against an absolute budget of 0.2);
all transposes are host-side so every HBM DMA is contiguous lines >= 4KB.
"""

import numpy as np

B, H, S, D = 4, 16, 4096, 128
T = 128          # block length (PE path matmul contraction dim)
KB = S // T      # 32 blocks
NG = 4           # blocks per matmul group (4*128 = 512 moving columns)
G = KB // NG     # 8 groups per pair
NCORES = 8
PAIRS = (B * H) // NCORES  # 8 pair-slots per core
NV = 5           # pairs fully on the Vector scan path
NPF = 2          # pairs fully on the Tensor (PE) path; +1 hybrid pair
S2 = S // 2
KB2 = KB // 2
G2 = G // 2

_nc_cache = {}


def _build_program():
    if "nc" in _nc_cache:
        return _nc_cache["nc"]

    import concourse.bass as bass
    import concourse.mybir as mybir
    from concourse.tile import TileContext

    f32 = mybir.dt.float32
    fp16 = mybir.dt.float16

    nc = bass.Bass(trn_type="TRN2")

    NP = NPF + 1  # PE coefficient slots: 2 full + 1 hybrid

    xv_d = nc.declare_dram_parameter("xv", [NV, D, S], fp16, isOutput=False)
    g_d = nc.declare_dram_parameter("g", [D, NV + 1], f32, isOutput=False)
    yv_d = nc.declare_dram_parameter("yv", [NV, D, S], fp16, isOutput=True)

    xp_d = nc.declare_dram_parameter("xp", [NPF, T, KB * D], fp16, isOutput=False)
    xph_d = nc.declare_dram_parameter("xph", [T, KB2 * D], fp16, isOutput=False)
    xvh_d = nc.declare_dram_parameter("xvh", [D, S2], fp16, isOutput=False)
    A_d = nc.declare_dram_parameter("A_all", [T, NP * T], fp16, isOutput=False)
    u_d = nc.declare_dram_parameter("u_all", [T, NP], fp16, isOutput=False)
    gv_d = nc.declare_dram_parameter("gv_all", [1, NP * T], fp16, isOutput=False)
    GT_d = nc.declare_dram_parameter("GT_all", [KB, NP * KB], fp16, isOutput=False)
    yp_d = nc.declare_dram_parameter("yp", [NPF, T, KB * D], fp16, isOutput=True)
    yph_d = nc.declare_dram_parameter("yph", [T, KB2 * D], fp16, isOutput=True)
    yvh_d = nc.declare_dram_parameter("yvh", [D, S2], fp16, isOutput=True)

    mult, add = mybir.AluOpType.mult, mybir.AluOpType.add

    with TileContext(nc) as tc:
        with (
            tc.tile_pool(name="const", bufs=1) as cpool,
            tc.tile_pool(name="xvin", bufs=3) as xvpool,
            tc.tile_pool(name="yvout", bufs=3) as yvpool,
            tc.tile_pool(name="xpin", bufs=3) as xppool,
            tc.tile_pool(name="ypout", bufs=2) as yppool,
            tc.tile_pool(name="rfl", bufs=2) as rfpool,
            tc.tile_pool(name="r32", bufs=2) as r32pool,
            tc.tile_pool(name="c32", bufs=2) as c32pool,
            tc.tile_pool(name="cfl", bufs=2) as cfpool,
            tc.tile_pool(name="grp_ps", bufs=2, space="PSUM") as gp_pool,
            tc.tile_pool(name="sum_ps", bufs=2, space="PSUM") as sp_pool,
            tc.tile_pool(name="c_ps", bufs=1, space="PSUM") as cp_pool,
        ):
            gc = cpool.tile([D, NV + 1], f32, tag="gc")
            nc.gpsimd.dma_start(out=gc[:], in_=g_d[:])
            Ac = cpool.tile([T, NP * T], fp16, tag="Ac")
            uc = cpool.tile([T, NP], fp16, tag="uc")
            gvc = cpool.tile([1, NP * T], fp16, tag="gvc")
            GTc = cpool.tile([KB, NP * KB], fp16, tag="GTc")
            nc.gpsimd.dma_start(out=Ac[:], in_=A_d[:])
            nc.gpsimd.dma_start(out=uc[:], in_=u_d[:])
            nc.gpsimd.dma_start(out=gvc[:], in_=gv_d[:])
            nc.gpsimd.dma_start(out=GTc[:], in_=GT_d[:])
            # hybrid boundary state, written in pe_phase_gt via scalar copy
            c7 = cpool.tile([D, 1], f32, tag="c7")

            v_state = {}
            pe_state = {}

            def v_scan(v):
                X = xvpool.tile([D, S], fp16, tag="Xv")
                nc.sync.dma_start(out=X[:], in_=xv_d[v])
                Y = yvpool.tile([D, S], fp16, tag="Yv")
                nc.vector.tensor_tensor_scan(
                    out=Y[:], data0=gc[:, v : v + 1].broadcast_to([D, S]),
                    data1=X[:], initial=0.0, op0=mult, op1=add,
                )
                v_state[v] = Y

            def v_out(v):
                Y = v_state.pop(v)
                nc.scalar.dma_start(out=yv_d[v], in_=Y[:])

            def v_scan_h(_):
                X = xvpool.tile([D, S2], fp16, tag="Xvh")
                nc.sync.dma_start(out=X[:], in_=xvh_d[:])
                Y = yvpool.tile([D, S2], fp16, tag="Yvh")
                nc.vector.tensor_tensor_scan(
                    out=Y[:], data0=gc[:, NV : NV + 1].broadcast_to([D, S2]),
                    data1=X[:], initial=c7[:, 0:1], op0=mult, op1=add,
                )
                v_state["h"] = Y

            def v_out_h(_):
                Y = v_state.pop("h")
                nc.scalar.dma_start(out=yvh_d[:], in_=Y[:])

            def pe_phase_a(q):
                """Block sums for a PE slot; q=2 is the half-length hybrid."""
                nk = KB if q < NPF else KB2
                ng = G if q < NPF else G2
                Xf = xppool.tile([T, KB * D], fp16, tag="Xp")
                X = Xf[:, 0 : nk * D]
                nc.sync.dma_start(
                    out=X, in_=xp_d[q] if q < NPF else xph_d[:]
                )
                Rff = rfpool.tile([1, KB * D], fp16, tag="Rf")
                Rflat = Rff[:, 0 : nk * D]
                for g in range(ng):
                    sl = slice(g * NG * D, (g + 1) * NG * D)
                    rp = sp_pool.tile([1, NG * D], f32, tag="rp")
                    nc.tensor.matmul(
                        rp[:], lhsT=uc[:, q : q + 1], rhs=X[:, sl],
                        start=True, stop=True,
                    )
                    nc.scalar.copy(out=Rflat[:, sl], in_=rp[:])
                R32f = r32pool.tile([KB, D], fp16, tag="R32")
                R32 = R32f[0:nk, :]
                nc.scalar.dma_start(out=R32, in_=Rflat)
                pe_state[q] = (X, R32)

            def pe_phase_gt(q):
                nk = KB if q < NPF else KB2
                X, R32 = pe_state.pop(q)
                cpf = cp_pool.tile([KB, D], f32, tag="cp")
                cp = cpf[0:nk, :]
                nc.tensor.matmul(
                    cp[:], lhsT=GTc[0:nk, q * KB : q * KB + nk], rhs=R32[:],
                    start=True, stop=True,
                )
                if q == NPF:
                    # boundary state y[S2-1, d] = sum_j gamma^(128(15-j)) r_j
                    # with R32 as the STATIONARY so the result lands as the
                    # [d, 1] column the DVE scan needs for `initial`
                    tp = cp_pool.tile([D, 1], f32, tag="tp")
                    nc.tensor.matmul(
                        tp[:], lhsT=R32[:],
                        rhs=GTc[0:nk, q * KB + nk : q * KB + nk + 1],
                        start=True, stop=True,
                    )
                    nc.scalar.copy(out=c7[:], in_=tp[:])
                C32f = c32pool.tile([KB, D], fp16, tag="C32")
                C32 = C32f[0:nk, :]
                nc.scalar.copy(out=C32, in_=cp[:])
                cff = cfpool.tile([1, KB * D], fp16, tag="cfh")
                cfh = cff[:, 0 : nk * D]
                nc.scalar.dma_start(out=cfh, in_=C32)
                pe_state[q] = (X, cfh)

            def pe_phase_scan(q):
                nk = KB if q < NPF else KB2
                ng = G if q < NPF else G2
                X, cfh = pe_state.pop(q)
                Ysf = yppool.tile([T, KB * D], fp16, tag="Ys")
                Ys = Ysf[:, 0 : nk * D]
                Aq = Ac[:, q * T : (q + 1) * T]
                gvq = gvc[:, q * T : (q + 1) * T]
                for g2 in range(ng // 2):
                    sl2 = slice(g2 * 2 * NG * D, (g2 + 1) * 2 * NG * D)
                    grp = gp_pool.tile([T, 2 * NG * D], f32, tag="grp")
                    for h in range(2):
                        g = g2 * 2 + h
                        nc.tensor.matmul(
                            grp[:, h * NG * D : (h + 1) * NG * D],
                            lhsT=Aq,
                            rhs=X[:, g * NG * D : (g + 1) * NG * D],
                            start=True, stop=False,
                            skip_group_check=True,
                        )
                    for h in range(2):
                        g = g2 * 2 + h
                        nc.tensor.matmul(
                            grp[:, h * NG * D : (h + 1) * NG * D],
                            lhsT=gvq,
                            rhs=cfh[:, g * NG * D : (g + 1) * NG * D],
                            start=False, stop=True,
                            skip_group_check=True,
                        )
                    nc.scalar.copy(out=Ys[:, sl2], in_=grp[:])
                    if g2 % 2 == 1:
                        slo = slice((g2 - 1) * 2 * NG * D, (g2 + 1) * 2 * NG * D)
                        # sync ring: idle once the x loads finish (~t=37)
                        if q < NPF:
                            nc.sync.dma_start(
                                out=yp_d[q, :, slo], in_=Ys[:, slo]
                            )
                        else:
                            nc.sync.dma_start(out=yph_d[:, slo], in_=Ys[:, slo])


            phases = {
                "vs": v_scan,
                "vo": v_out,
                "vsh": v_scan_h,
                "voh": v_out_h,
                "a": pe_phase_a,
                "gt": pe_phase_gt,
                "sc": pe_phase_scan,
            }
            for kind, idx in [
                ("vs", 0), ("a", 2), ("a", 0), ("vs", 1), ("a", 1),
                ("gt", 2), ("vs", 2), ("vo", 0), ("sc", 2), ("gt", 0),
                ("vs", 3), ("vo", 1), ("sc", 0), ("gt", 1), ("vo", 2),
                ("vs", 4), ("sc", 1), ("vo", 3), ("vsh", 0), ("voh", 0),
                ("vo", 4),
            ]:
                phases[kind](idx)

    # Walrus allows 1 sync wait on engine instructions / 2 on DMAs; move
    # excess waits onto InstEventSemaphore carriers.
    import bass_rust

    bass_rust.generate_event_semaphores(nc)

    _nc_cache["nc"] = nc
    return nc


def _pe_constants(g):
    """fp16 coefficient tables from float64 gamma powers."""
    pw = np.power(g, np.arange(S, dtype=np.float64))
    t_idx = np.arange(T)
    t_minus_s = t_idx[None, :] - t_idx[:, None]
    A = np.where(t_minus_s >= 0, pw[np.clip(t_minus_s, 0, None)], 0.0)
    u = pw[127 - t_idx]
    gv = pw[t_idx + 1]
    pw128 = np.power(pw[T], np.arange(KB, dtype=np.float64))
    k_minus_j = np.arange(KB)[None, :] - 1 - np.arange(KB)[:, None]
    GT = np.where(k_minus_j >= 0, pw128[np.clip(k_minus_j, 0, None)], 0.0)
    return A, u, gv, GT


def _pe_layout(xpair):
    """[S', D] f32 -> [T, (block, d)] fp16 scan layout."""
    nk = xpair.shape[0] // T
    return (
        xpair.reshape(nk, T, D).transpose(1, 0, 2).reshape(T, nk * D)
        .astype(np.float16)
    )


def _make_in_maps(tensor, gamma):
    x = np.asarray(tensor, dtype=np.float32).reshape(B * H, S, D)
    gam = np.asarray(gamma, dtype=np.float32).reshape(H)
    NP = NPF + 1

    in_maps = []
    for c in range(NCORES):
        pids = [c * PAIRS + p for p in range(PAIRS)]
        xv = np.empty((NV, D, S), np.float16)
        gcol = np.empty((D, NV + 1), np.float32)
        for v in range(NV):
            pid = pids[v]
            xv[v] = x[pid].T.astype(np.float16)
            gcol[:, v] = gam[pid % H]
        hid = pids[NV + NPF]  # the hybrid pair
        gcol[:, NV] = gam[hid % H]
        xp = np.empty((NPF, T, KB * D), np.float16)
        A_all = np.empty((T, NP * T), np.float16)
        u_all = np.empty((T, NP), np.float16)
        gv_all = np.empty((1, NP * T), np.float16)
        GT_all = np.zeros((KB, NP * KB), np.float16)
        for q in range(NP):
            pid = pids[NV + q]
            if q < NPF:
                xp[q] = _pe_layout(x[pid])
            A, u, gv, GT = _pe_constants(float(gam[pid % H]))
            A_all[:, q * T : (q + 1) * T] = A.astype(np.float16)
            u_all[:, q] = u.astype(np.float16)
            gv_all[0, q * T : (q + 1) * T] = gv.astype(np.float16)
            GT_all[:, q * KB : (q + 1) * KB] = GT.astype(np.float16)
            if q == NPF:
                # boundary-state weights: y[S2-1] = sum_j g^(128(15-j)) r_j
                gq = np.float64(gam[pid % H])
                w16 = np.power(
                    np.power(gq, T), KB2 - 1 - np.arange(KB2, dtype=np.float64)
                )
                GT_all[0:KB2, q * KB + KB2] = w16.astype(np.float16)
        in_maps.append(
            {
                "xv": xv,
                "g": gcol,
                "xp": xp,
                "xph": _pe_layout(x[hid][:S2]),
                "xvh": x[hid][S2:].T.astype(np.float16),
                "A_all": A_all,
                "u_all": u_all,
                "gv_all": gv_all,
                "GT_all": GT_all,
            }
        )
    return in_maps


def _gather_results(results):
    """Per-core result dicts -> full [B, H, S, D] f32 output."""
    y = np.empty((B * H, S, D), np.float32)
    for c in range(NCORES):
        r = results[c]
        yv = np.asarray(r["yv"])
        for v in range(NV):
            y[c * PAIRS + v] = yv[v].T
        yp = np.asarray(r["yp"])
        for q in range(NPF):
            y[c * PAIRS + NV + q] = (
                yp[q].reshape(T, KB, D).transpose(1, 0, 2).reshape(S, D)
            )
        hid = c * PAIRS + NV + NPF
        y[hid, :S2] = (
            np.asarray(r["yph"]).reshape(T, KB2, D).transpose(1, 0, 2)
            .reshape(S2, D)
        )
        y[hid, S2:] = np.asarray(r["yvh"]).T
    return y.reshape(B, H, S, D)


def kernel(tensor, gamma):
    from concourse.bass_utils import run_bass_kernel_spmd

    in_maps = _make_in_maps(tensor, gamma)
    nc = _build_program()
    res = run_bass_kernel_spmd(nc, in_maps, list(range(NCORES))).results
    return _gather_results(res)


# revision 67
# speedup vs baseline: 1.0853x; 1.0388x over previous
"""Discounted cumsum (y[b,h,t,d] = x[b,h,t,d] + gamma[h] * y[b,h,t-1,d]) on 8 trn2 cores.

Pure data parallelism: 64 (b,h) pairs, 8 per core. The per-pair recurrence runs
on two engines (the DVE scan instruction is fixed at ~2 cycles/elem, so Vector
alone caps at ~69 us/core; PE matmuls run ~1.1 ns/column under the hardware
power throttle, so a PE pair costs ~2x a DVE pair):

- 5 pairs fully on the Vector engine: layout [d=128 partitions, s=4096 free];
  the whole recurrence is ONE tensor_tensor_scan (state = gamma*state + x,
  fp32 internal state). gamma stays EXACT fp32 via a stride-0 broadcast AP -
  a rounded gamma would be raised to the power t by the recurrence, amplifying
  its error by ~t, while independently-rounded coefficient TABLES (PE path)
  and fp16 data are fine at the 2e-2 gate.

- 2 pairs fully on the Tensor engine (blocked parallel scan, single fp16):
  layout [t-within-block=128 partitions, 32 blocks x 128 d free]. Block sums
  r_k from u=gamma^(127-s) matmuls, carries C_k from one gamma^128-power
  matmul, carry injection as K=1 matmuls of gamma^(t+1) x C accumulated onto
  the within-block scan matmuls A[s,t]=gamma^(t-s) in PSUM. The scan matmuls
  go FIRST in each PSUM bank (start=True resets the whole bank region) and
  both scans of a bank precede the carry-dependent injects so a late carry
  never stalls them.

- 1 HYBRID pair: PE computes the first S/2 with the blocked scan (it needs no
  cross-engine input); the boundary state y[S/2-1] is computed from the block
  sums by one extra matmul with R32 as the STATIONARY operand (so the result
  lands directly as the [d,1] column a DVE scan needs for `initial`), and the
  second half runs as a final half-length DVE scan. This rebalances 5.5
  DVE-pair-equivalents vs 2.5 PE, which is the measured-rate optimum.

Engine streams execute in program order and a DMA instruction's semaphore
wait stalls both its engine and its ring FIFO, so phases are emitted in an
order that keeps every stream stall-free:
- sync ring: x loads, then the PE-path outputs (the ring is idle once the
  loads drain and the PE outputs become ready in that window).
- scalar ring: the block-sum scatter + carry gather transposes (enqueued the
  moment their data is ready) and the Vector-path outputs.
- gpsimd ring: constants only (its software DGE has ~6+ us latency).
- PE stream: each pair's block sums are emitted before the previous pair's
  carry-dependent half, so the in-order tensor stream always has work while
  a carry chain resolves (the scatter/gather semaphore round trip is ~8 us).

I/O is fp16 both ways (absmax error ~1e-2 against an absolute budget of 0.2);
all transposes are host-side so every HBM DMA is contiguous lines >= 4KB.
"""

import numpy as np

B, H, S, D = 4, 16, 4096, 128
T = 128          # block length (PE path matmul contraction dim)
KB = S // T      # 32 blocks
NG = 4           # blocks per matmul group (4*128 = 512 moving columns)
G = KB // NG     # 8 groups per pair
NCORES = 8
PAIRS = (B * H) // NCORES  # 8 pair-slots per core
NV = 5           # pairs fully on the Vector scan path
NPF = 2          # pairs fully on the Tensor (PE) path; +1 hybrid pair
S2 = S // 2
KB2 = KB // 2
G2 = G // 2

_nc_cache = {}


def _build_program():
    if "nc" in _nc_cache:
        return _nc_cache["nc"]

    import concourse.bass as bass
    import concourse.mybir as mybir
    from concourse.tile import TileContext

    f32 = mybir.dt.float32
    fp16 = mybir.dt.float16

    nc = bass.Bass(trn_type="TRN2")

    NP = NPF + 1  # PE coefficient slots: 2 full + 1 hybrid

    xv_d = nc.declare_dram_parameter("xv", [NV, D, S], fp16, isOutput=False)
    g_d = nc.declare_dram_parameter("g", [D, NV + 1], f32, isOutput=False)
    yv_d = nc.declare_dram_parameter("yv", [NV, D, S], fp16, isOutput=True)

    xp_d = nc.declare_dram_parameter("xp", [NPF, T, KB * D], fp16, isOutput=False)
    xph_d = nc.declare_dram_parameter("xph", [T, KB2 * D], fp16, isOutput=False)
    xvh_d = nc.declare_dram_parameter("xvh", [D, S2], fp16, isOutput=False)
    A_d = nc.declare_dram_parameter("A_all", [T, NP * T], fp16, isOutput=False)
    u_d = nc.declare_dram_parameter("u_all", [T, NP], fp16, isOutput=False)
    gv_d = nc.declare_dram_parameter("gv_all", [1, NP * T], fp16, isOutput=False)
    GT_d = nc.declare_dram_parameter("GT_all", [KB, NP * KB], fp16, isOutput=False)
    yp_d = nc.declare_dram_parameter("yp", [NPF, T, KB * D], fp16, isOutput=True)
    yph_d = nc.declare_dram_parameter("yph", [T, KB2 * D], fp16, isOutput=True)
    yvh_d = nc.declare_dram_parameter("yvh", [D, S2], fp16, isOutput=True)

    mult, add = mybir.AluOpType.mult, mybir.AluOpType.add

    with TileContext(nc) as tc:
        with (
            tc.tile_pool(name="const", bufs=1) as cpool,
            tc.tile_pool(name="xvin", bufs=3) as xvpool,
            tc.tile_pool(name="yvout", bufs=3) as yvpool,
            tc.tile_pool(name="xpin", bufs=3) as xppool,
            tc.tile_pool(name="ypout", bufs=2) as yppool,
            tc.tile_pool(name="rfl", bufs=2) as rfpool,
            tc.tile_pool(name="r32", bufs=2) as r32pool,
            tc.tile_pool(name="c32", bufs=2) as c32pool,
            tc.tile_pool(name="cfl", bufs=2) as cfpool,
            tc.tile_pool(name="grp_ps", bufs=2, space="PSUM") as gp_pool,
            tc.tile_pool(name="sum_ps", bufs=2, space="PSUM") as sp_pool,
            tc.tile_pool(name="c_ps", bufs=1, space="PSUM") as cp_pool,
        ):
            gc = cpool.tile([D, NV + 1], f32, tag="gc")
            nc.gpsimd.dma_start(out=gc[:], in_=g_d[:])
            Ac = cpool.tile([T, NP * T], fp16, tag="Ac")
            uc = cpool.tile([T, NP], fp16, tag="uc")
            gvc = cpool.tile([1, NP * T], fp16, tag="gvc")
            GTc = cpool.tile([KB, NP * KB], fp16, tag="GTc")
            nc.gpsimd.dma_start(out=Ac[:], in_=A_d[:])
            nc.gpsimd.dma_start(out=uc[:], in_=u_d[:])
            nc.gpsimd.dma_start(out=gvc[:], in_=gv_d[:])
            nc.gpsimd.dma_start(out=GTc[:], in_=GT_d[:])
            # hybrid boundary state, written in pe_phase_gt via scalar copy
            c7 = cpool.tile([D, 1], f32, tag="c7")

            v_state = {}
            pe_state = {}

            def v_scan(v, split_load=False):
                X = xvpool.tile([D, S], fp16, tag="Xv")
                if split_load:
                    # first load: both rings are empty at t~9, so halves on
                    # different rings land ~2us sooner and shift the whole
                    # serial DVE chain left
                    nc.sync.dma_start(out=X[:, :S2], in_=xv_d[v][:, :S2])
                    nc.scalar.dma_start(out=X[:, S2:], in_=xv_d[v][:, S2:])
                else:
                    nc.sync.dma_start(out=X[:], in_=xv_d[v])
                Y = yvpool.tile([D, S], fp16, tag="Yv")
                nc.vector.tensor_tensor_scan(
                    out=Y[:], data0=gc[:, v : v + 1].broadcast_to([D, S]),
                    data1=X[:], initial=0.0, op0=mult, op1=add,
                )
                v_state[v] = Y

            def v_out(v):
                Y = v_state.pop(v)
                nc.scalar.dma_start(out=yv_d[v], in_=Y[:])

            def v_scan_h(_):
                X = xvpool.tile([D, S2], fp16, tag="Xvh")
                nc.sync.dma_start(out=X[:], in_=xvh_d[:])
                Y = yvpool.tile([D, S2], fp16, tag="Yvh")
                nc.vector.tensor_tensor_scan(
                    out=Y[:], data0=gc[:, NV : NV + 1].broadcast_to([D, S2]),
                    data1=X[:], initial=c7[:, 0:1], op0=mult, op1=add,
                )
                v_state["h"] = Y

            def v_out_h(_):
                Y = v_state.pop("h")
                nc.scalar.dma_start(out=yvh_d[:], in_=Y[:])

            def pe_phase_a(q):
                """Block sums for a PE slot; q=2 is the half-length hybrid."""
                nk = KB if q < NPF else KB2
                ng = G if q < NPF else G2
                Xf = xppool.tile([T, KB * D], fp16, tag="Xp")
                X = Xf[:, 0 : nk * D]
                nc.sync.dma_start(
                    out=X, in_=xp_d[q] if q < NPF else xph_d[:]
                )
                Rff = rfpool.tile([1, KB * D], fp16, tag="Rf")
                Rflat = Rff[:, 0 : nk * D]
                for g in range(ng):
                    sl = slice(g * NG * D, (g + 1) * NG * D)
                    rp = sp_pool.tile([1, NG * D], f32, tag="rp")
                    nc.tensor.matmul(
                        rp[:], lhsT=uc[:, q : q + 1], rhs=X[:, sl],
                        start=True, stop=True,
                    )
                    nc.scalar.copy(out=Rflat[:, sl], in_=rp[:])
                R32f = r32pool.tile([KB, D], fp16, tag="R32")
                R32 = R32f[0:nk, :]
                nc.scalar.dma_start(out=R32, in_=Rflat)
                pe_state[q] = (X, R32)

            def pe_phase_gt(q):
                nk = KB if q < NPF else KB2
                X, R32 = pe_state.pop(q)
                cpf = cp_pool.tile([KB, D], f32, tag="cp")
                cp = cpf[0:nk, :]
                nc.tensor.matmul(
                    cp[:], lhsT=GTc[0:nk, q * KB : q * KB + nk], rhs=R32[:],
                    start=True, stop=True,
                )
                if q == NPF:
                    # boundary state y[S2-1, d] = sum_j gamma^(128(15-j)) r_j
                    # with R32 as the STATIONARY so the result lands as the
                    # [d, 1] column the DVE scan needs for `initial`
                    tp = cp_pool.tile([D, 1], f32, tag="tp")
                    nc.tensor.matmul(
                        tp[:], lhsT=R32[:],
                        rhs=GTc[0:nk, q * KB + nk : q * KB + nk + 1],
                        start=True, stop=True,
                    )
                    nc.scalar.copy(out=c7[:], in_=tp[:])
                C32f = c32pool.tile([KB, D], fp16, tag="C32")
                C32 = C32f[0:nk, :]
                nc.scalar.copy(out=C32, in_=cp[:])
                cff = cfpool.tile([1, KB * D], fp16, tag="cfh")
                cfh = cff[:, 0 : nk * D]
                nc.scalar.dma_start(out=cfh, in_=C32)
                pe_state[q] = (X, cfh)

            def pe_phase_scan(q):
                nk = KB if q < NPF else KB2
                ng = G if q < NPF else G2
                X, cfh = pe_state.pop(q)
                Ysf = yppool.tile([T, KB * D], fp16, tag="Ys")
                Ys = Ysf[:, 0 : nk * D]
                Aq = Ac[:, q * T : (q + 1) * T]
                gvq = gvc[:, q * T : (q + 1) * T]
                for g2 in range(ng // 2):
                    sl2 = slice(g2 * 2 * NG * D, (g2 + 1) * 2 * NG * D)
                    grp = gp_pool.tile([T, 2 * NG * D], f32, tag="grp")
                    for h in range(2):
                        g = g2 * 2 + h
                        nc.tensor.matmul(
                            grp[:, h * NG * D : (h + 1) * NG * D],
                            lhsT=Aq,
                            rhs=X[:, g * NG * D : (g + 1) * NG * D],
                            start=True, stop=False,
                            skip_group_check=True,
                        )
                    for h in range(2):
                        g = g2 * 2 + h
                        nc.tensor.matmul(
                            grp[:, h * NG * D : (h + 1) * NG * D],
                            lhsT=gvq,
                            rhs=cfh[:, g * NG * D : (g + 1) * NG * D],
                            start=False, stop=True,
                            skip_group_check=True,
                        )
                    nc.scalar.copy(out=Ys[:, sl2], in_=grp[:])
                    if g2 % 2 == 1:
                        slo = slice((g2 - 1) * 2 * NG * D, (g2 + 1) * 2 * NG * D)
                        # sync ring: idle once the x loads finish (~t=37)
                        if q < NPF:
                            nc.sync.dma_start(
                                out=yp_d[q, :, slo], in_=Ys[:, slo]
                            )
                        else:
                            nc.sync.dma_start(out=yph_d[:, slo], in_=Ys[:, slo])


            phases = {
                "vs": v_scan,
                "vo": v_out,
                "vsh": v_scan_h,
                "voh": v_out_h,
                "a": pe_phase_a,
                "gt": pe_phase_gt,
                "sc": pe_phase_scan,
            }
            for kind, idx, *kw in [
                ("vs", 0, {"split_load": True}), ("a", 2), ("a", 0),
                ("vs", 1), ("a", 1), ("gt", 2), ("vs", 2), ("vo", 0),
                ("sc", 2), ("gt", 0), ("vs", 3), ("vo", 1), ("sc", 0),
                ("gt", 1), ("vo", 2), ("vs", 4), ("sc", 1), ("vo", 3),
                ("vsh", 0), ("voh", 0), ("vo", 4),
            ]:
                phases[kind](idx, **(kw[0] if kw else {}))

    # Walrus allows 1 sync wait on engine instructions / 2 on DMAs; move
    # excess waits onto InstEventSemaphore carriers.
    import bass_rust

    bass_rust.generate_event_semaphores(nc)

    _nc_cache["nc"] = nc
    return nc


def _pe_constants(g):
    """fp16 coefficient tables from float64 gamma powers."""
    pw = np.power(g, np.arange(S, dtype=np.float64))
    t_idx = np.arange(T)
    t_minus_s = t_idx[None, :] - t_idx[:, None]
    A = np.where(t_minus_s >= 0, pw[np.clip(t_minus_s, 0, None)], 0.0)
    u = pw[127 - t_idx]
    gv = pw[t_idx + 1]
    pw128 = np.power(pw[T], np.arange(KB, dtype=np.float64))
    k_minus_j = np.arange(KB)[None, :] - 1 - np.arange(KB)[:, None]
    GT = np.where(k_minus_j >= 0, pw128[np.clip(k_minus_j, 0, None)], 0.0)
    return A, u, gv, GT


def _pe_layout(xpair):
    """[S', D] f32 -> [T, (block, d)] fp16 scan layout."""
    nk = xpair.shape[0] // T
    return (
        xpair.reshape(nk, T, D).transpose(1, 0, 2).reshape(T, nk * D)
        .astype(np.float16)
    )


def _make_in_maps(tensor, gamma):
    x = np.asarray(tensor, dtype=np.float32).reshape(B * H, S, D)
    gam = np.asarray(gamma, dtype=np.float32).reshape(H)
    NP = NPF + 1

    in_maps = []
    for c in range(NCORES):
        pids = [c * PAIRS + p for p in range(PAIRS)]
        xv = np.empty((NV, D, S), np.float16)
        gcol = np.empty((D, NV + 1), np.float32)
        for v in range(NV):
            pid = pids[v]
            xv[v] = x[pid].T.astype(np.float16)
            gcol[:, v] = gam[pid % H]
        hid = pids[NV + NPF]  # the hybrid pair
        gcol[:, NV] = gam[hid % H]
        xp = np.empty((NPF, T, KB * D), np.float16)
        A_all = np.empty((T, NP * T), np.float16)
        u_all = np.empty((T, NP), np.float16)
        gv_all = np.empty((1, NP * T), np.float16)
        GT_all = np.zeros((KB, NP * KB), np.float16)
        for q in range(NP):
            pid = pids[NV + q]
            if q < NPF:
                xp[q] = _pe_layout(x[pid])
            A, u, gv, GT = _pe_constants(float(gam[pid % H]))
            A_all[:, q * T : (q + 1) * T] = A.astype(np.float16)
            u_all[:, q] = u.astype(np.float16)
            gv_all[0, q * T : (q + 1) * T] = gv.astype(np.float16)
            GT_all[:, q * KB : (q + 1) * KB] = GT.astype(np.float16)
            if q == NPF:
                # boundary-state weights: y[S2-1] = sum_j g^(128(15-j)) r_j
                gq = np.float64(gam[pid % H])
                w16 = np.power(
                    np.power(gq, T), KB2 - 1 - np.arange(KB2, dtype=np.float64)
                )
                GT_all[0:KB2, q * KB + KB2] = w16.astype(np.float16)
        in_maps.append(
            {
                "xv": xv,
                "g": gcol,
                "xp": xp,
                "xph": _pe_layout(x[hid][:S2]),
                "xvh": x[hid][S2:].T.astype(np.float16),
                "A_all": A_all,
                "u_all": u_all,
                "gv_all": gv_all,
                "GT_all": GT_all,
            }
        )
    return in_maps


def _gather_results(results):
    """Per-core result dicts -> full [B, H, S, D] f32 output."""
    y = np.empty((B * H, S, D), np.float32)
    for c in range(NCORES):
        r = results[c]
        yv = np.asarray(r["yv"])
        for v in range(NV):
            y[c * PAIRS + v] = yv[v].T
        yp = np.asarray(r["yp"])
        for q in range(NPF):
            y[c * PAIRS + NV + q] = (
                yp[q].reshape(T, KB, D).transpose(1, 0, 2).reshape(S, D)
            )
        hid = c * PAIRS + NV + NPF
        y[hid, :S2] = (
            np.asarray(r["yph"]).reshape(T, KB2, D).transpose(1, 0, 2)
            .reshape(S2, D)
        )
        y[hid, S2:] = np.asarray(r["yvh"]).T
    return y.reshape(B, H, S, D)


def kernel(tensor, gamma):
    from concourse.bass_utils import run_bass_kernel_spmd

    in_maps = _make_in_maps(tensor, gamma)
    nc = _build_program()
    res = run_bass_kernel_spmd(nc, in_maps, list(range(NCORES))).results
    return _gather_results(res)
